# revision 3
# baseline (speedup 1.0000x reference)
"""EngagementBiasedMHA on 8 Trainium2 NeuronCores.

Sharding: 4 batches x 2 head-groups (8 heads each).  Each core computes, for
its (batch, head-group):
  - K^T projection in [feat, token] layout and V projection in [token, feat]
    layout (phase 1); V is stored per key-tile as [ones(64) | V_h] so the PV
    matmul also produces the softmax denominator on partitions 0:64
  - per 512-query chunk: Q^T projection (overlapped with attention of the
    previous chunk), then attention in transposed layout: S^T = K @ Q^T with
    keys on partitions, so the per-key engagement bias/mask folds into the
    Exp activation as a per-partition bias, and exp(S^T) is already the
    correct (lhs-contraction) layout for the PV matmul
  - O^T = Vhat^T @ P^T accumulated over key tiles (rows 0:64 = replicated
    softmax denominator, rows 64:128 = head output)
  - row-parallel partial output projection in transposed form:
    y^T = out_w[hg-rows].T-tiles @ O-tiles, bias added per-partition on ACT
Matmul operands are bf16 (4x PE throughput vs fp32); accumulation stays fp32.

exp is split across two engines so the Scalar engine no longer paces the
attention loop: even key tiles use the ACT table exp; odd key tiles use a
one-instruction DVE "fast exp2" (Schraudolph): the Q projection is pre-scaled
by 128*log2(e)/8 on the host, so exp(S/8+bk) ~= bf16_bits(round(S' + bk'))
where bk' = 128*log2(e)*bk + 16256 - 128*sigma; the int16-cast write (RNE,
clamped at 0 for masked keys) lands directly on the bf16 grid.

Host side: transpose/slice/pack inputs per core (large-row DMA layouts), then
sum the two partial outputs per batch and transpose (row-parallel unshard).
"""

import sys

if "/opt/trn_rl_repo" not in sys.path:
    sys.path.insert(0, "/opt/trn_rl_repo")

import numpy as np
from concourse import bacc, tile
import concourse.mybir as mybir
from concourse.bass_utils import run_bass_kernel_spmd

F32 = mybir.dt.float32
BF16 = mybir.dt.bfloat16
I16 = mybir.dt.int16
NP_BF16 = mybir.dt.np(BF16)
AF = mybir.ActivationFunctionType
ALU = mybir.AluOpType

B, T, D, H = 4, 2048, 1024, 16
HD = 64
HG = 8           # heads per core
NKT = T // 128   # 16 key/token tiles
NQC = T // 512   # 4 query chunks
NDT = D // 128   # 8 d_in tiles
VROW = HG * 128  # 1024 Vhat columns per key tile: per head [ones(64) | V(64)]

LOG2E = 1.4426950408889634
SQ = float(128.0 * LOG2E * 0.125)      # Q pre-scale = 23.083...
SIGMA = 0.0430                          # Schraudolph shift (minimax-ish)
EXP_C = float(16256.0 - 128.0 * SIGMA)  # bf16 bias constant
ACT_SCALE = float(0.125 / SQ)           # undo the Q pre-scale inside ACT exp
# key tiles whose exp runs on DVE (rest on ACT); roughly balances both engines
DVE_KT = frozenset((1, 3, 5, 7, 9, 11, 13))

_cache = {}

# Results of the most recent run (for the test harness to read exec times).
last_results = None


def _build_program():
    nc = bacc.Bacc("TRN2", target_bir_lowering=False, debug=False, num_devices=8)
    # packed DMA-friendly layouts (8-16KB contiguous rows per partition)
    xc0_d = nc.declare_dram_parameter("xc0", [128, NDT * 512], BF16, isOutput=False)
    xcr_d = nc.declare_dram_parameter("xcr", [128, NDT * 1536], BF16, isOutput=False)
    # wqk2 row p, col mi*1024 + d*128 + f  with mi order [K0..K3, Q0..Q3]
    wqk_d = nc.declare_dram_parameter("wqk", [128, 8 * 1024], BF16, isOutput=False)
    wv_d = nc.declare_dram_parameter("wv", [128, NDT * 512], BF16, isOutput=False)
    wo_d = nc.declare_dram_parameter("wo", [128, 4 * 1024], BF16, isOutput=False)
    # smalls blob: [BQK 8 | BV 512 | ENG 16 | MSK 16 | BO 8] = 560 cols
    sml_d = nc.declare_dram_parameter("sml", [128, 560], F32, isOutput=False)
    y_d = nc.declare_dram_parameter("y", [D, T], F32, isOutput=True)

    with tile.TileContext(nc) as tc:
        with (
            tc.tile_pool(name="persist", bufs=1) as persist,
            tc.tile_pool(name="wpool", bufs=1) as wpool,
            tc.tile_pool(name="small", bufs=1) as small,
            tc.tile_pool(name="ptpool", bufs=4) as ptpool,
            tc.tile_pool(name="otpool", bufs=9) as otpool,
            tc.tile_pool(name="evacpool", bufs=3) as evacpool,
            tc.tile_pool(name="recpool", bufs=3) as recpool,
            tc.tile_pool(name="psmix", bufs=4, space="PSUM") as psmix,
            tc.tile_pool(name="psST", bufs=2, space="PSUM") as psST,
        ):
            # ---- resident activations / weights (bf16) ----
            XT = persist.tile([128, NDT * T], BF16, name="XT")
            WQK = persist.tile([128, 8 * 1024], BF16, name="WQK")
            WV = wpool.tile([128, NDT * 512], BF16, name="WV", tag="wv_wo")
            WO = wpool.tile([128, 4 * 1024], BF16, name="WO", tag="wv_wo")
            SML = small.tile([128, 560], F32, name="SML")

            # ---- prologue DMA: critical tensors first, issue spread over
            # four engine queues so descriptor issue isn't serialized ----
            nc.sync.dma_start(SML[:], sml_d[:])
            xt3 = XT[:].rearrange("p (d t) -> p d t", t=T)
            for i in range(4):
                lo = i * 32
                nc.sync.dma_start(
                    xt3[lo:lo + 32, :, 0:512],
                    xc0_d[lo:lo + 32, :].rearrange("p (d t) -> p d t", t=512))
            # K-half of WQK (cols 4096:8192 of SBUF tile <- dram cols 0:4096)
            for i in range(2):
                lo = i * 64
                nc.sync.dma_start(WQK[lo:lo + 64, 4096:8192], wqk_d[lo:lo + 64, 0:4096])
            for i in range(2):
                lo = i * 64
                nc.scalar.dma_start(WV[lo:lo + 64, :], wv_d[lo:lo + 64, :])
            for i in range(4):
                lo = i * 32
                nc.gpsimd.dma_start(
                    xt3[lo:lo + 32, :, 512:T],
                    xcr_d[lo:lo + 32, :].rearrange("p (d t) -> p d t", t=1536))
            for i in range(2):
                lo = i * 64
                nc.scalar.dma_start(WQK[lo:lo + 64, 0:4096], wqk_d[lo:lo + 64, 4096:8192])
            for i in range(2):
                lo = i * 64
                nc.scalar.dma_start(WO[lo:lo + 64, :], wo_d[lo:lo + 64, :])

            BQK = SML[:, 0:8]
            BV = SML[:, 8:520]
            ENG = SML[:, 520:536]
            MSK = SML[:, 536:552]
            BO = SML[:, 552:560]

            # ---- per-key bias: BK = ln(max(eng, 1e-6)) - 1e9 * mask ----
            BK = small.tile([128, NKT], F32, name="BK")
            nc.vector.tensor_scalar_max(BK[:], ENG[:], 1e-6)
            nc.scalar.activation(BK[:], BK[:], AF.Ln)
            MK9 = small.tile([128, NKT], F32, name="MK9")
            nc.vector.tensor_scalar_mul(MK9[:], MSK[:], -1e9)
            nc.vector.tensor_add(BK[:], BK[:], MK9[:])
            # DVE fast-exp bias: BK2 = 128*log2e*BK + (16256 - 128*sigma)
            BK2 = small.tile([128, NKT], F32, name="BK2")
            nc.vector.tensor_scalar(BK2[:], BK[:], float(128.0 * LOG2E), EXP_C,
                                    ALU.mult, ALU.add)

            QTKT = persist.tile([128, 8 * T], BF16, name="QTKT")
            VHAT = persist.tile([128, NKT * VROW], BF16, name="VHAT")
            nc.gpsimd.memset(VHAT[:], 1.0)

            # ---- phase 1: K^T and V projections (chunked over tokens) ----
            for c in range(NQC):
                for m in range(4, 8):  # K feature tiles
                    ps = psmix.tile([128, 512], F32, name="ps_k", tag="mix")
                    for d in range(NDT):
                        nc.tensor.matmul(
                            ps[:],
                            lhsT=WQK[:, m * 1024 + d * 128: m * 1024 + (d + 1) * 128],
                            rhs=XT[:, d * T + c * 512: d * T + c * 512 + 512],
                            start=(d == 0), stop=(d == NDT - 1),
                        )
                    nc.scalar.activation(
                        QTKT[:, m * T + c * 512: m * T + c * 512 + 512],
                        ps[:], AF.Identity, bias=BQK[:, m:m + 1])
                for t4 in range(4):
                    t = c * 4 + t4
                    ps = psmix.tile([128, 512], F32, name="ps_v", tag="mix")
                    for d in range(NDT):
                        nc.tensor.matmul(
                            ps[:],
                            lhsT=XT[:, d * T + t * 128: d * T + (t + 1) * 128],
                            rhs=WV[:, d * 512:(d + 1) * 512],
                            start=(d == 0), stop=(d == NDT - 1),
                        )
                    vslice = VHAT[:, t * VROW:(t + 1) * VROW].rearrange(
                        "p (h c) -> p h c", c=128)[:, :, 64:128]
                    nc.vector.tensor_add(
                        vslice,
                        ps[:].rearrange("p (h c) -> p h c", c=64),
                        BV[:].rearrange("p (h c) -> p h c", c=64))

            # ---- phase 2: per query chunk: attention with Q-proj of the next
            # chunk and out-proj of the previous chunk interleaved into the
            # per-head-pair slack ----
            def q_proj(qc2, m):
                ps = psmix.tile([128, 512], F32, name="ps_q", tag="mix")
                for d in range(NDT):
                    nc.tensor.matmul(
                        ps[:],
                        lhsT=WQK[:, m * 1024 + d * 128: m * 1024 + (d + 1) * 128],
                        rhs=XT[:, d * T + qc2 * 512: d * T + qc2 * 512 + 512],
                        start=(d == 0), stop=(d == NDT - 1),
                    )
                nc.scalar.activation(
                    QTKT[:, m * T + qc2 * 512: m * T + qc2 * 512 + 512],
                    ps[:], AF.Identity, bias=BQK[:, m:m + 1])

            def out_proj(qc2, otc2, ct):
                # y^T tile: [128 out-cols, 512 tokens]; bias per-partition
                ps = psmix.tile([128, 512], F32, name="ps_y", tag="mix")
                for f in range(4):
                    nc.tensor.matmul(
                        ps[:],
                        lhsT=WO[:, f * 1024 + ct * 128: f * 1024 + ct * 128 + 128],
                        rhs=otc2[f][:],
                        start=(f == 0), stop=(f == 3))
                yv = evacpool.tile([128, 512], F32, name="yv", tag="yv")
                if ct % 2 == 0:
                    nc.scalar.activation(yv[:], ps[:], AF.Identity,
                                         bias=BO[:, ct:ct + 1])
                else:
                    nc.vector.tensor_scalar_add(yv[:], ps[:], BO[:, ct:ct + 1])
                nc.sync.dma_start(
                    y_d[ct * 128:(ct + 1) * 128, qc2 * 512:(qc2 + 1) * 512], yv[:])

            for m in range(4):
                q_proj(0, m)
            prev = None  # (qc, otc) awaiting out-projection
            for qc in range(NQC):
                otc = []
                for hp in range(4):
                    qt = hp
                    ktf = 4 + hp
                    op0 = psmix.tile([128, 512], F32, name="op0", tag="mix")
                    op1 = psmix.tile([128, 512], F32, name="op1", tag="mix")
                    ops = (op0, op1)
                    for kt in range(NKT):
                        st = psST.tile([128, 1024], F32, name="st", tag="st")
                        for sub in range(2):
                            lo = sub * 64
                            nc.tensor.matmul(
                                st[:, sub * 512:(sub + 1) * 512],
                                lhsT=QTKT[lo:lo + 64, ktf * T + kt * 128: ktf * T + (kt + 1) * 128],
                                rhs=QTKT[lo:lo + 64, qt * T + qc * 512: qt * T + qc * 512 + 512],
                                start=True, stop=True)
                        pt = ptpool.tile([128, 1024], BF16, name="pt", tag="pt")
                        if kt in DVE_KT:
                            nc.vector.tensor_scalar(
                                pt[:].bitcast(I16), st[:], BK2[:, kt:kt + 1], 0.0,
                                ALU.add, ALU.max)
                        else:
                            nc.scalar.activation(
                                pt[:], st[:], AF.Exp,
                                bias=BK[:, kt:kt + 1], scale=ACT_SCALE)
                        for sub in range(2):
                            h = 2 * hp + sub
                            nc.tensor.matmul(
                                ops[sub][:],
                                lhsT=VHAT[:, kt * VROW + h * 128: kt * VROW + (h + 1) * 128],
                                rhs=pt[:, sub * 512:(sub + 1) * 512],
                                start=(kt == 0), stop=(kt == NKT - 1))
                    # evacuate raw accumulators quickly to free the PSUM slots,
                    # normalize from SBUF afterwards
                    OTc = otpool.tile([128, 512], BF16, name="OTc", tag="otc")
                    for sub in range(2):
                        rec = recpool.tile([64, 512], F32, name="rec", tag="rec")
                        nc.vector.reciprocal_approx_fast(rec[:], ops[sub][0:64, :])
                        nc.vector.tensor_mul(
                            OTc[sub * 64:sub * 64 + 64, :],
                            ops[sub][64:128, :], rec[:])
                    otc.append(OTc)
                    # boundary work in the engine slack after this head pair
                    if prev is not None:
                        out_proj(prev[0], prev[1], 2 * hp)
                        out_proj(prev[0], prev[1], 2 * hp + 1)
                    if qc + 1 < NQC:
                        q_proj(qc + 1, hp)
                prev = (qc, otc)
            for ct in range(8):
                out_proj(prev[0], prev[1], ct)
    nc.compile()
    return nc


def get_program():
    if "nc" not in _cache:
        _cache["nc"] = _build_program()
    return _cache["nc"]


def shard_inputs(x, engagement, mask, qkv_w, qkv_b, out_w, out_b):
    """Build the per-core input maps (host-side layout prep only)."""
    x = np.asarray(x, dtype=np.float32)
    engagement = np.asarray(engagement, dtype=np.float32)
    maskf = np.asarray(mask).astype(np.float32)
    qkv_w = np.asarray(qkv_w, dtype=np.float32)
    qkv_b = np.asarray(qkv_b, dtype=np.float32)
    out_w = np.asarray(out_w, dtype=np.float32)
    out_b = np.asarray(out_b, dtype=np.float32)

    qkvT = qkv_w.T  # [D, 3D]
    outT = out_w.T  # [D, D]
    in_maps = []
    for cix in range(8):
        b, hg = cix // 2, cix % 2
        qcols = qkvT[:, hg * 512:(hg + 1) * 512] * np.float32(SQ)
        kcols = qkvT[:, 1024 + hg * 512: 1024 + (hg + 1) * 512]
        sel = np.concatenate([qcols, kcols], axis=1)  # [1024 din, 1024 feats]
        # [d, p, m, f] -> [m, p, d, f]; row-major pack [p, (mi d f)] with
        # mi order = [K0..3, Q0..3] so the K half is dram cols 0:4096
        wq4 = sel.reshape(NDT, 128, 8, 128).transpose(2, 1, 0, 3)  # [m, p, d, f]
        morder = [4, 5, 6, 7, 0, 1, 2, 3]
        wqk2 = wq4[morder].transpose(1, 0, 2, 3).reshape(128, 8 * 1024)
        bq = qkv_b[hg * 512:(hg + 1) * 512].reshape(4, 128).T * np.float32(SQ)
        bk = qkv_b[1024 + hg * 512: 1024 + (hg + 1) * 512].reshape(4, 128).T
        bo = (out_b.reshape(8, 128).T if hg == 0
              else np.zeros((128, 8), np.float32))
        # x^T packed d-major: [p, d, t]
        xt3 = x[b].T.reshape(NDT, 128, T).transpose(1, 0, 2)  # [p, d, t]
        xc0 = xt3[:, :, 0:512].reshape(128, NDT * 512)
        xcr = xt3[:, :, 512:T].reshape(128, NDT * 1536)
        # wv packed d-major [p, (d f)]
        wv2 = qkvT[:, 2048 + hg * 512: 2048 + (hg + 1) * 512].reshape(
            NDT, 128, 512).transpose(1, 0, 2).reshape(128, NDT * 512)
        # wo packed f-major [p, (f cols)]
        wo2 = outT[hg * 512:(hg + 1) * 512, :].reshape(
            4, 128, 1024).transpose(1, 0, 2).reshape(128, 4 * 1024)
        sml = np.concatenate([
            np.concatenate([bq, bk], axis=1),                       # 8
            np.broadcast_to(qkv_b[2048 + hg * 512: 2048 + (hg + 1) * 512],
                            (128, 512)),                            # 512
            engagement[b].reshape(NKT, 128).T,                      # 16
            maskf[b].reshape(NKT, 128).T,                           # 16
            bo,                                                     # 8
        ], axis=1).astype(np.float32)
        in_maps.append({
            "xc0": np.ascontiguousarray(xc0).astype(NP_BF16),
            "xcr": np.ascontiguousarray(xcr).astype(NP_BF16),
            "wqk": np.ascontiguousarray(wqk2).astype(NP_BF16),
            "wv": np.ascontiguousarray(wv2).astype(NP_BF16),
            "wo": np.ascontiguousarray(wo2).astype(NP_BF16),
            "sml": np.ascontiguousarray(sml),
        })
    return in_maps


def kernel(x, engagement, mask, qkv_w, qkv_b, out_w, out_b):
    global last_results
    nc = get_program()
    in_maps = shard_inputs(x, engagement, mask, qkv_w, qkv_b, out_w, out_b)
    res = run_bass_kernel_spmd(nc, in_maps, list(range(8)))
    last_results = res
    out = np.empty((B, T, D), dtype=np.float32)
    for b in range(B):
        out[b] = (res.results[2 * b]["y"] + res.results[2 * b + 1]["y"]).T
    return out
